# revision 21
# baseline (speedup 1.0000x reference)
"""GameTheoreticAttention Trainium2 kernel — linear-attention formulation.

Full inputs in, full output out. 8-way shard = 2 batches x 4 head-pairs;
core c handles batch n=c//4, heads {2j, 2j+1} (j=c%4), embed cols
[128j, 128j+128).

The attention logits here are ~2e-8 (payoff probs ~1/L shrink q/k by
~2.4e-4 each), so exp(x) = 1 + x to f32 rounding and softmax-attention
collapses exactly to a rank-65 linear form:

  out[q,:] = (Svw + pq[q] * (q[q] @ M')) / Z[q]
  M' = sum_k kw_k (x) vw_k / sqrt(E)   [64x64 per head]
  Svw = sum_k vw_k,  Z[q] = L + pq[q]*(q[q] @ sum_k kw_k)/sqrt(E)

Z's q-dependent part is ~4e-9 relative, below f32 resolution of 1/Z
(= 2.44140625e-4 for every q on hardware), so Z = L exactly in f32.
Verified vs the jax reference: rel err ~6e-7 end to end. Per core:
  - q payoff scores via a tiny PE matmul + ACT exp (no M dependency, runs
    during the k/v phase); A-row = pq/L built in-place on the score rows.
  - k/v payoff probs from row-layout tiles (DVE cube multiply + reduce,
    ACT exp, gpsimd partition all-reduce); pk/sqrt(E) folded into kw.
  - slots interleaved (s = 2t+h) so one [128 kw-dims] x [130 vw-dims]
    matmul per k-tile accumulates BOTH heads' M' (32 matmuls total);
    vr ones-columns make kbar fall out, Svw comes from a DVE slot-reduce
    + partition all-reduce + one K=1 transpose matmul.
  - A-rows broadcast across partitions with stride-0-source DMAs.
  - out^T chunks: psU = blockdiag(M') @ qT, on = psU*A_bc + Svw/L (DVE
    halves + ACT per-partition bias), then row-parallel fc_out (128-row
    slice of w_out^T); partial [4096, 512] streams out over 3 queues.
Host sums the 4 partials per batch and adds b_out.
"""

import os
import sys

for _p in ("/root/.axon_site", "/root/.axon_site/_ro/trn_rl_repo", "/opt/trn_rl_repo"):
    if os.path.isdir(_p) and _p not in sys.path:
        sys.path.append(_p)

import ml_dtypes
import numpy as np

import concourse.bass as bass  # noqa: E402,F401
import concourse.tile as tile  # noqa: E402
from concourse import bacc, bass_isa, mybir  # noqa: E402
from concourse.bass_utils import run_bass_kernel_spmd  # noqa: E402

F32 = mybir.dt.float32
BF16 = mybir.dt.bfloat16
X = mybir.AxisListType.X
MULT = mybir.AluOpType.mult
ADD = mybir.AluOpType.add
EXP = mybir.ActivationFunctionType.Exp
ACOPY = mybir.ActivationFunctionType.Copy
AIDENT = mybir.ActivationFunctionType.Identity
BF = ml_dtypes.bfloat16

EMBED = 512
HD = 64
N = 2
L = 4096
NCORES = 8
NCH = 8  # 512-wide q chunks
NKT = 32  # 128-tall k tiles per head
INV_SQRT_E = float(1.0 / np.sqrt(512.0))

# fallback switch if stride-0-source DMA broadcast is rejected
DMA_BCAST = True


def build_program(debug=False):
    nc = bacc.Bacc("TRN2", target_bir_lowering=False, debug=False)

    qT_d = nc.dram_tensor("qT", [128, L], BF16, kind="ExternalInput").ap()
    kr_d = nc.dram_tensor("kr", [128, 64, 64], BF16, kind="ExternalInput").ap()
    vr_d = nc.dram_tensor("vr", [128, 64, 65], BF16, kind="ExternalInput").ap()
    wt_d = nc.dram_tensor("wt", [128, EMBED], BF16, kind="ExternalInput").ap()
    wq2_d = nc.dram_tensor("wq2", [128, 2], BF16, kind="ExternalInput").ap()
    wkb_d = nc.dram_tensor("wkb", [128, 64], BF16, kind="ExternalInput").ap()
    wvb_d = nc.dram_tensor("wvb", [128, 64], BF16, kind="ExternalInput").ap()
    y_d = nc.dram_tensor("y", [L, EMBED], BF16, kind="ExternalOutput").ap()
    if debug:
        dbg = {
            "dkr": nc.dram_tensor("dkr", [128, 64, 64], BF16, kind="ExternalOutput").ap(),
            "dvr": nc.dram_tensor("dvr", [128, 64, 65], BF16, kind="ExternalOutput").ap(),
            "dstatU": nc.dram_tensor("dstatU", [128, 128], BF16, kind="ExternalOutput").ap(),
            "dsvcol": nc.dram_tensor("dsvcol", [128, 1], F32, kind="ExternalOutput").ap(),
            "dab": nc.dram_tensor("dab", [128, 512], F32, kind="ExternalOutput").ap(),
            "dON": nc.dram_tensor("dON", [128, L], BF16, kind="ExternalOutput").ap(),
            "dpsU": nc.dram_tensor("dpsU", [128, 512], F32, kind="ExternalOutput").ap(),
        }

    with tile.TileContext(nc) as tc:
        with (
            tc.tile_pool(name="persist", bufs=1) as persist,
            tc.tile_pool(name="prod", bufs=2) as prod_pool,
            tc.tile_pool(name="o1", bufs=4) as o1_pool,
            tc.tile_pool(name="on", bufs=4) as on_pool,
            tc.tile_pool(name="ysb", bufs=6) as y_pool,
            tc.tile_pool(name="ps_q", bufs=1, space="PSUM") as ps_q_pool,
            tc.tile_pool(name="ps_m", bufs=1, space="PSUM") as ps_m_pool,
            tc.tile_pool(name="ps_u", bufs=2, space="PSUM") as ps_u_pool,
            tc.tile_pool(name="ps_y", bufs=4, space="PSUM") as ps_y_pool,
        ):
            def ptile(shape, tag, dt=F32):
                return persist.tile(shape, dt, tag=tag, name=tag)

            qT = ptile([128, L], "qT_sb", BF16)
            kr = ptile([128, 64, 64], "kr_sb", BF16)
            vr = ptile([128, 64, 65], "vr_sb", BF16)
            wt_sb = ptile([128, EMBED], "wt_sb", BF16)
            wq2_sb = ptile([128, 2], "wq2_sb", BF16)
            wkb_sb = ptile([128, 64], "wkb_sb", BF16)
            wvb_sb = ptile([128, 64], "wvb_sb", BF16)
            statU = ptile([128, 128], "statU", BF16)
            statQ = ptile([128, 33], "statQ", BF16)
            svp = ptile([128, 128], "svp")
            svpr = ptile([128, 128], "svpr")
            svrow = ptile([1, 128], "svrow", BF16)
            svcol = ptile([128, 1], "svcol")
            svL = ptile([128, 1], "svL")
            ones1 = ptile([1, 1], "ones1", BF16)
            esq = [ptile([1, L], f"esq{h}") for h in range(2)]
            zpq = [ptile([1, NCH], f"zpq{h}") for h in range(2)]
            zq = [ptile([1, 1], f"zq{h}") for h in range(2)]
            zqi = [ptile([1, 1], f"zqi{h}") for h in range(2)]
            ones64 = ptile([1, 64], "ones64", BF16)
            zqrow = ptile([1, 128], "zqrow", BF16)
            zql = ptile([128, 1], "zql")

            # ---- loads: qT first on its own queue (q-scores start early)
            nc.scalar.dma_start(qT[:], qT_d[:])
            nc.sync.dma_start(wq2_sb[:], wq2_d[:])
            nc.sync.dma_start(wkb_sb[:], wkb_d[:])
            nc.sync.dma_start(kr[:], kr_d[:])
            nc.gpsimd.dma_start(wvb_sb[:], wvb_d[:])
            nc.gpsimd.dma_start(vr[:], vr_d[:])
            nc.gpsimd.dma_start(wt_sb[:], wt_d[:])

            nc.vector.memset(ones1[:], 1.0)
            nc.vector.memset(ones64[:], 1.0)
            nc.gpsimd.memset(statU[:], 0.0)
            nc.gpsimd.memset(statQ[:], 0.0)

            # ---- q payoff scores (PE+ACT, no M dependency): rows 0 / 32
            nc.vector.tensor_copy(statQ[:, 0:1], wq2_sb[:, 0:1])
            nc.vector.tensor_copy(statQ[:, 32:33], wq2_sb[:, 1:2])
            for jc in range(NCH):
                cs = slice(512 * jc, 512 * (jc + 1))
                psq = ps_q_pool.tile([33, 512], F32, tag="ps_q", name=f"psq{jc}")
                nc.tensor.matmul(psq[:], statQ[:], qT[:, cs], start=True, stop=True)
                for h in range(2):
                    nc.scalar.activation(
                        esq[h][:, cs],
                        psq[32 * h : 32 * h + 1, :],
                        EXP,
                        accum_out=zpq[h][:, jc : jc + 1],
                    )

            # ---- payoff probs for k and v (row layout, slots s = 2t+h)
            def payoff(r3, wb, extra_scale, tag, eng):
                r = r3[:, :, 0:64]
                prod = prod_pool.tile(
                    [128, 64, 64], BF16, tag="prod", name=f"prod_{tag}"
                )
                eng.tensor_tensor(
                    prod[:],
                    r,
                    wb[:].unsqueeze(1).broadcast_to([128, 64, 64]),
                    op=MULT,
                )
                scol = ptile([128, 64], f"scol_{tag}")
                nc.vector.reduce_sum(scol[:].unsqueeze(2), prod[:], axis=X)
                ecol = ptile([128, 64], f"ecol_{tag}")
                nc.scalar.activation(ecol[:], scol[:], EXP)
                ep = ptile([128, 2], f"ep_{tag}")
                for h in range(2):
                    nc.vector.reduce_sum(ep[:, h : h + 1], ecol[:, h::2], axis=X)
                zs = ptile([128, 2], f"zs_{tag}")
                nc.gpsimd.partition_all_reduce(
                    zs[:], ep[:], channels=128, reduce_op=bass_isa.ReduceOp.add
                )
                zi = ptile([128, 2], f"zi_{tag}")
                nc.vector.reciprocal_approx_fast(zi[:], zs[:])
                pcol = ptile([128, 64], f"pcol_{tag}")
                for h in range(2):
                    if extra_scale is None:
                        eng.tensor_scalar_mul(
                            pcol[:, h::2], ecol[:, h::2], zi[:, h : h + 1]
                        )
                    else:
                        eng.tensor_scalar(
                            pcol[:, h::2],
                            ecol[:, h::2],
                            zi[:, h : h + 1],
                            extra_scale,
                            op0=MULT,
                            op1=MULT,
                        )
                eng.tensor_tensor(
                    r, r, pcol[:].unsqueeze(2).broadcast_to([128, 64, 64]), op=MULT
                )

            payoff(kr, wkb_sb, INV_SQRT_E, "k", nc.vector)
            payoff(vr, wvb_sb, None, "v", nc.gpsimd)
            if debug:
                nc.sync.dma_start(dbg["dkr"][:], kr[:])
                nc.sync.dma_start(dbg["dvr"][:], vr[:])

            # ---- M'' both heads per k-tile: psM [128, 130]
            psM = ps_m_pool.tile([128, 130], F32, tag="ps_m", name="psM")
            for t in range(NKT):
                nc.tensor.matmul(
                    psM[:],
                    kr[:, 2 * t : 2 * t + 2, :],
                    vr[:, 2 * t : 2 * t + 2, :],
                    start=(t == 0),
                    stop=(t == NKT - 1),
                    skip_group_check=True,
                )

            # ---- Svw: reduce vw over slots (DVE) + all-reduce over
            # partitions (gpsimd) + K=1 transpose matmul -> [128, 1]
            for h in range(2):
                nc.vector.reduce_sum(
                    svp[:, 64 * h : 64 * h + 64].unsqueeze(2),
                    vr[:, h::2, 0:64].transpose([0, 2, 1]),
                    axis=X,
                )
            nc.gpsimd.partition_all_reduce(
                svpr[:], svp[:], channels=128, reduce_op=bass_isa.ReduceOp.add
            )
            nc.scalar.copy(svrow[:], svpr[0:1, :])
            psSv = ps_q_pool.tile([128, 1], F32, tag="ps_q", name="psSv")
            nc.tensor.matmul(psSv[:], svrow[:], ones1[:], start=True, stop=True)
            nc.vector.tensor_copy(svcol[:], psSv[:])
            nc.scalar.activation(svL[:], svcol[:], ACOPY, scale=float(1.0 / L))

            # ---- statU blockdiag from psM
            nc.scalar.copy(statU[0:64, 0:64], psM[0:64, 0:64])
            nc.scalar.copy(statU[64:128, 64:128], psM[64:128, 65:129])
            if debug:
                nc.sync.dma_start(dbg["dstatU"][:], statU[:])
                nc.sync.dma_start(dbg["dsvcol"][:], svcol[:])

            # ---- zql[d] = (1/zq[h(d)])/L as a per-partition column via a
            # tiny blockrow + K=1 transpose matmul (A-scale folds into the
            # on-activation's scale operand; esq rows broadcast raw)
            for h in range(2):
                nc.vector.reduce_sum(zq[h][:], zpq[h][:], axis=X)
                nc.vector.reciprocal_approx_fast(zqi[h][:], zq[h][:])
                nc.vector.tensor_scalar(
                    zqrow[:, 64 * h : 64 * h + 64],
                    ones64[:],
                    zqi[h][:],
                    float(1.0 / L),
                    op0=MULT,
                    op1=MULT,
                )
            psZ = ps_q_pool.tile([128, 1], F32, tag="ps_q", name="psZ")
            nc.tensor.matmul(psZ[:], zqrow[:], ones1[:], start=True, stop=True)
            nc.vector.tensor_copy(zql[:], psZ[:])

            # ---- broadcast exp rows across 64 partitions per head-half
            ab = [ptile([64, L], f"ab{h}") for h in range(2)]
            for h in range(2):
                nc.gpsimd.partition_broadcast(ab[h][:], esq[h][:], channels=64)

            # ---- main loop: psU = blockdiag(M) @ qT; on = psU*A + Svw/L;
            # fc_out row-parallel; stream y out on rotating queues
            dma_engines = [nc.sync, nc.gpsimd, nc.scalar]
            for jc in range(NCH):
                cs = slice(512 * jc, 512 * (jc + 1))
                psU = ps_u_pool.tile([128, 512], F32, tag="ps_u", name=f"psU{jc}")
                nc.tensor.matmul(psU[:], statU[:], qT[:, cs], start=True, stop=True)
                o1t = o1_pool.tile([128, 512], F32, tag="o1", name=f"o1_{jc}")
                for h in range(2):
                    nc.vector.tensor_tensor(
                        o1t[64 * h : 64 * h + 64, :],
                        psU[64 * h : 64 * h + 64, :],
                        ab[h][:, cs],
                        op=MULT,
                    )
                on = on_pool.tile([128, 512], BF16, tag="on", name=f"on_{jc}")
                nc.scalar.activation(
                    on[:], o1t[:], AIDENT, scale=zql[:], bias=svL[:]
                )
                if debug:
                    nc.sync.dma_start(dbg["dON"][:, cs], on[:])
                    if jc == 0:
                        stg = o1_pool.tile([128, 512], F32, tag="o1", name="dbg_psU")
                        nc.vector.tensor_copy(stg[:], psU[:])
                        nc.sync.dma_start(dbg["dpsU"][:], stg[:])
                        nc.sync.dma_start(dbg["dab"][0:64, :], ab[0][:, 0:512])
                        nc.sync.dma_start(dbg["dab"][64:128, :], ab[1][:, 0:512])
                for qq in range(4):
                    psY = ps_y_pool.tile(
                        [128, 512], F32, tag="ps_y", name=f"psY_{jc}_{qq}"
                    )
                    nc.tensor.matmul(
                        psY[:],
                        on[:, 128 * qq : 128 * (qq + 1)],
                        wt_sb[:],
                        start=True,
                        stop=True,
                    )
                    y_sb = y_pool.tile(
                        [128, 512], BF16, tag="y_sb", name=f"y_{jc}_{qq}"
                    )
                    if qq % 2 == 0:
                        nc.scalar.copy(y_sb[:], psY[:])
                    else:
                        nc.vector.tensor_copy(y_sb[:], psY[:])
                    r0 = (4 * jc + qq) * 128
                    eng = dma_engines[(4 * jc + qq) % 3]
                    eng.dma_start(y_d[r0 : r0 + 128, :], y_sb[:])

    nc.compile()
    return nc


_NC = None


def _get_nc():
    global _NC
    if _NC is None:
        _NC = build_program()
    return _NC


def _pack_rows(v, ones_col):
    """[L, 128] f32 -> [128, 64, 64(+1)] bf16 with interleaved slots:
    out[p, 2t+h, d] = v[128t+p, 64h+d]; optional ones column at d=64."""
    w = 65 if ones_col else 64
    out = np.ones((128, 64, w), np.float32)
    vr = v.reshape(NKT, 128, 2, 64).transpose(1, 0, 2, 3)  # p t h d
    out[:, :, 0:64] = vr.reshape(128, 64, 64)
    return out.astype(BF)


def make_in_maps(values, keys, query, w_vp, w_kp, w_qp, w_out):
    values = np.ascontiguousarray(values, np.float32)
    keys = np.ascontiguousarray(keys, np.float32)
    query = np.ascontiguousarray(query, np.float32)
    w_vp = np.asarray(w_vp, np.float32)
    w_kp = np.asarray(w_kp, np.float32)
    w_qp = np.asarray(w_qp, np.float32)
    w_out = np.asarray(w_out, np.float32)

    wq2 = np.zeros((128, 2), np.float32)
    wq2[0:64, 0] = w_qp
    wq2[64:128, 1] = w_qp
    wq2 = wq2.astype(BF)
    wkb = np.tile(w_kp[None, :], (128, 1)).astype(BF)
    wvb = np.tile(w_vp[None, :], (128, 1)).astype(BF)
    wt_full = np.ascontiguousarray(w_out.T)  # [e_in, e_out]

    in_maps = []
    for c in range(NCORES):
        n, j = divmod(c, 4)
        e0 = j * 128
        in_maps.append(
            {
                "qT": np.ascontiguousarray(query[n, :, e0 : e0 + 128].T).astype(BF),
                "kr": _pack_rows(keys[n, :, e0 : e0 + 128], False),
                "vr": _pack_rows(values[n, :, e0 : e0 + 128], True),
                "wt": np.ascontiguousarray(wt_full[e0 : e0 + 128, :]).astype(BF),
                "wq2": wq2,
                "wkb": wkb,
                "wvb": wvb,
            }
        )
    return in_maps


def assemble(results, b_out):
    out = np.zeros((N, L, EMBED), np.float32)
    for c in range(NCORES):
        out[c // 4] += results[c]["y"].astype(np.float32)
    out += np.asarray(b_out, np.float32)[None, None, :]
    return out


def kernel(values, keys, query, w_vp, w_kp, w_qp, w_out, b_out):
    nc = _get_nc()
    in_maps = make_in_maps(values, keys, query, w_vp, w_kp, w_qp, w_out)
    res = run_bass_kernel_spmd(nc, in_maps, core_ids=list(range(NCORES)))
    return assemble(res.results, b_out)
